# revision 1
# baseline (speedup 1.0000x reference)
"""BatchMultiHeadGraphAttention TRN2 kernel.

Reference computation (per batch b, head h):
    h_prime = h[b] @ w[h]                          # [n, f]
    t = tanh(h_prime)
    src = t @ a_src[h];  dst = t @ a_dst[h]        # [n]
    s[i, j] = leaky_relu(src[i] + dst[j], 0.2)
    s = where(adj[b] | eye, s, -inf)
    attn = softmax(s, axis=-1)
    out[b, h] = attn @ h_prime + bias

Sharding: 8 cores, one (b, h) slab per core (bs=4 x H=2).

Per-core design (all n-indices on chip live in a permuted order sigma
induced by the u16-pair xbar transpose of adj; sigma is applied
consistently to j (partitions of S^T / rows of h_prime) and i (free dim
of S^T / output rows), so the diagonal stays the diagonal and only the
final output DMA un-permutes):

  - hT via PE transposes of h (fp32r), h_prime^sigma = hT @ w on PE (fp32r)
  - tanh on ACT from the h_prime PSUM (full fp32), src/dst dots via DVE
    tensor_tensor_reduce against broadcast a_src/a_dst
  - scores built transposed S^T[j, i]: one DVE scalar_tensor_tensor per
    tile: (adjT_u8 * 200) + (src_bcast - 200)  [mask folded additively],
    diagonal (adj|eye) fixed by copy_predicated with an identity mask
  - leaky-relu+exp fused as P = max(exp(s), exp(0.2 s)) (2 ACT passes,
    max on gpsimd); dst added via the ACT per-partition bias
  - PV on PE in fp32r with a ones-column producing Z (softmax denom);
    final 1/Z scaling fused into the PSUM->SBUF eviction copy.
"""

import numpy as np

BS, N, H, F_IN, F_OUT = 4, 2048, 2, 768, 768
NCORES = 8
L = 200.0  # mask offset; exp(0.2*(s-L)) <= e^-37 -> negligible

_CACHE = {}


def _build(has_bias: bool):
    import os
    import concourse.bass as bass
    import concourse.mybir as mybir
    import concourse.tile as tile
    from concourse import bacc
    from concourse.masks import make_identity

    dt = mybir.dt
    AF = mybir.ActivationFunctionType
    OP = mybir.AluOpType

    NT = N // 128            # 16 n-tiles
    KT = F_IN // 128         # 6 k-tiles
    NG = 4                   # i groups
    BB = (N // 256) // NG    # 256-blocks per group
    GW = N // NG             # 1024 group width
    CHG = GW // 128          # 8 i-chunks per group

    nc = bacc.Bacc("TRN2", target_bir_lowering=False, debug=False,
                   num_devices=NCORES)

    d_h = nc.dram_tensor("h", [N, F_IN], dt.float32, kind="ExternalInput")
    d_adj = nc.dram_tensor("adj", [N, N], dt.uint8, kind="ExternalInput")
    d_w = nc.dram_tensor("w", [F_IN, F_OUT], dt.float32, kind="ExternalInput")
    d_asrc = nc.dram_tensor("a_src", [F_OUT], dt.float32, kind="ExternalInput")
    d_adst = nc.dram_tensor("a_dst", [F_OUT], dt.float32, kind="ExternalInput")
    if has_bias:
        d_bias = nc.dram_tensor("bias", [F_OUT], dt.float32,
                                kind="ExternalInput")
    d_out = nc.dram_tensor("out", [N, F_OUT], dt.float32,
                           kind="ExternalOutput")

    def sig_off(tau):
        # on-chip position m = 128*tau + p  <->  logical n index
        # sigma(m) = 256*(tau//2) + 2*p + (tau % 2)
        return 256 * (tau // 2) + (tau % 2)

    with tile.TileContext(nc) as tc:
        with tc.tile_pool(name="const", bufs=1) as cpool, \
             tc.tile_pool(name="persist", bufs=1) as pp:
            # ---- constants ----
            c200 = cpool.tile([128, 1], dt.float32, tag="c200")
            nc.gpsimd.memset(c200[:], L)
            ones_col = cpool.tile([128, 1], dt.float32, tag="ones_col")
            nc.gpsimd.memset(ones_col[:], 1.0)
            ident = cpool.tile([128, 128], dt.float32, tag="ident")
            make_identity(nc, ident[:])
            eye_u8 = cpool.tile([128, 128], dt.uint8, tag="eye_u8")
            nc.vector.tensor_copy(eye_u8[:], ident[:])

            # ---- persistent buffers ----
            if has_bias:
                bias_row = cpool.tile([1, F_OUT], dt.float32, tag="bias_row")
                nc.sync.dma_start(bias_row[:],
                                  d_bias.ap().rearrange("(o f) -> o f", o=1))
                bias_bc = pp.tile([128, F_OUT], dt.float32, tag="bias_bc")
                nc.gpsimd.partition_broadcast(bias_bc[:], bias_row[:])

            # h_prime^sigma tiles [128, F_OUT+1] f32r (ones col at F_OUT)
            hp = [pp.tile([128, F_OUT + 2], dt.float32r, tag=f"hp{t}", name=f"hp{t}")
                  for t in range(NT)]
            src_col = pp.tile([128, NT], dt.float32, tag="src_col")
            dst_col = pp.tile([128, NT], dt.float32, tag="dst_col")
            dst02_col = pp.tile([128, NT], dt.float32, tag="dst02_col")

            # adj staging: 8 tiles [128, N] u16 (xbar transpose of u16
            # pairs) -- issued first so the strided xbar DMAs overlap
            # phases 1/2
            stag = [pp.tile([128, N], dt.uint16, tag=f"stag{t}", name=f"stag{t}")
                    for t in range(NT // 2)]
            adj16 = d_adj.ap().bitcast(dt.uint16)       # [N, N//2]

            kphase = int(os.environ.get("KPHASE", "9"))
            # ---- phase 1: hT = transpose(h), via PE (fp32r) ----
            with tc.tile_pool(name="ph1", bufs=6) as hpool, \
                 tc.tile_pool(name="ph1t", bufs=1) as htp, \
                 tc.tile_pool(name="ph1ps", bufs=4, space="PSUM") as psum1:
                # w as f32r [k][128, F_OUT]  (phase 1/2 scoped)
                wr = [htp.tile([128, F_OUT], dt.float32r, tag=f"wr{k}", name=f"wr{k}")
                      for k in range(KT)]
                for k in range(KT):
                    wtmp = hpool.tile([128, F_OUT], dt.float32, tag="hin",
                                      name=f"wtmp{k}")
                    nc.sync.dma_start(wtmp[:], d_w[128 * k:128 * (k + 1), :])
                    nc.scalar.copy(wr[k][:], wtmp[:])
                # a_src/a_dst broadcast [128, F_OUT]
                asrc_row = htp.tile([1, F_OUT], dt.float32, tag="asrc_row")
                nc.sync.dma_start(asrc_row[:],
                                  d_asrc.ap().rearrange("(o f) -> o f", o=1))
                adst_row = htp.tile([1, F_OUT], dt.float32, tag="adst_row")
                nc.sync.dma_start(adst_row[:],
                                  d_adst.ap().rearrange("(o f) -> o f", o=1))
                asrc_bc = htp.tile([128, F_OUT], dt.float32, tag="asrc_bc")
                nc.gpsimd.partition_broadcast(asrc_bc[:], asrc_row[:])
                adst_bc = htp.tile([128, F_OUT], dt.float32, tag="adst_bc")
                nc.gpsimd.partition_broadcast(adst_bc[:], adst_row[:])

                hT = [htp.tile([128, N], dt.float32r, tag=f"hT{k}", name=f"hT{k}")
                      for k in range(KT)]
                if kphase >= 1:
                    for ngrp in range(NT // 4):
                        hr = []
                        for nn in range(4):
                            t = 4 * ngrp + nn
                            ht_in = hpool.tile([128, F_IN], dt.float32, tag="hin")
                            nc.sync.dma_start(ht_in[:],
                                              d_h[128 * t:128 * (t + 1), :])
                            hr.append(ht_in)
                        for k in range(KT):
                            ps = psum1.tile([128, 512], dt.float32, tag="tps")
                            for nn in range(4):
                                nc.tensor.transpose(
                                    ps[:, 128 * nn:128 * (nn + 1)],
                                    hr[nn][:, 128 * k:128 * (k + 1)],
                                    ident[:])
                            # store hT in sigma-permuted column order so
                            # phase-2 lhsT slices are contiguous:
                            # in pos 256*t + 2*q + o -> out pos 256*t+128*o+q
                            psperm = ps[:].rearrange(
                                "p (t q o) -> p t o q", t=2, q=128, o=2)
                            nc.vector.tensor_copy(
                                hT[k][:, 512 * ngrp:512 * (ngrp + 1)], psperm)

                if kphase == 1:
                    nc.sync.dma_start(d_out[0:128, 0:F_OUT],
                                      hT[0][:, 0:F_OUT].bitcast(dt.float32))
                if kphase == 0:
                    nc.sync.dma_start(d_out[0:128, 0:F_OUT],
                                      wr[0][:, 0:F_OUT].bitcast(dt.float32))

                # adj xbar transposes issued after the h/w loads so they
                # don't hog the HWDGE rings at kernel start
                for t in range(NT // 2):
                    nc.sync.dma_start(stag[t][:],
                                      adj16[:, 128 * t:128 * (t + 1)],
                                      transpose=True)

                if kphase >= 2:
                    # ---- phase 2: h_prime^sigma = hT_sigma.T @ w ----
                    with tc.tile_pool(name="ph2", bufs=2) as tpool, \
                         tc.tile_pool(name="ph2ps", bufs=2, space="PSUM") as psum2:
                        for tau in range(NT):

                            ps = psum2.tile([128, F_OUT], dt.float32, tag="hpps")
                            for k in range(KT):
                                lhsT = hT[k][:, 128 * tau:128 * (tau + 1)]
                                nc.tensor.matmul(ps[:, 0:512], lhsT,
                                                 wr[k][:, 0:512],
                                                 start=(k == 0), stop=(k == KT - 1))
                                nc.tensor.matmul(ps[:, 512:F_OUT], lhsT,
                                                 wr[k][:, 512:F_OUT],
                                                 start=(k == 0), stop=(k == KT - 1))
                            # evict to f32r (+ ones column)
                            kp2 = os.environ.get("KP2", "")
                            nc.scalar.copy(hp[tau][:, 0:F_OUT], ps[:])
                            if "noones" not in kp2:
                                nc.scalar.copy(hp[tau][:, F_OUT:F_OUT + 1],
                                               ones_col[:])
                                nc.scalar.mul(hp[tau][:, F_OUT + 1:F_OUT + 2],
                                              ones_col[:], 0.0)
                            if has_bias:
                                nc.gpsimd.tensor_tensor(
                                    hp[tau][:, 0:F_OUT],
                                    hp[tau][:, 0:F_OUT].bitcast(dt.float32),
                                    bias_bc[:], op=OP.add)
                            # tanh (full fp32 from PSUM) + src/dst dots
                            if "nodots" in kp2:
                                continue
                            tnh = tpool.tile([128, F_OUT], dt.float32, tag="tnh")
                            nc.scalar.activation(tnh[:], hp[tau][:, 0:F_OUT].bitcast(dt.float32), AF.Tanh)
                            if "nottr" in kp2:
                                continue
                            scr = tpool.tile([128, F_OUT], dt.float32, tag="scr")
                            acc1 = tpool.tile([128, 1], dt.float32, tag="acc1")
                            nc.vector.scalar_tensor_tensor(
                                scr[:], tnh[:], ones_col[:], asrc_bc[:],
                                op0=OP.mult, op1=OP.mult,
                                accum_out=acc1[:])
                            nc.vector.tensor_copy(src_col[:, tau:tau + 1],
                                                  acc1[:])
                            scr2 = tpool.tile([128, F_OUT], dt.float32, tag="scr")
                            acc2 = tpool.tile([128, 1], dt.float32, tag="acc2")
                            nc.vector.scalar_tensor_tensor(
                                scr2[:], tnh[:], ones_col[:], adst_bc[:],
                                op0=OP.mult, op1=OP.mult,
                                accum_out=acc2[:])
                            nc.vector.tensor_copy(dst_col[:, tau:tau + 1],
                                                  acc2[:])

            if kphase == 2:
                nc.sync.dma_start(d_out[0:128, 0:F_OUT],
                                  hp[0][:, 0:F_OUT].bitcast(dt.float32))

            if kphase >= 3:
                # ---- phase 3: src row assembly + broadcasts ----
                nc.vector.tensor_scalar_mul(dst02_col[:], dst_col[:], 0.2)
                with tc.tile_pool(name="ph3", bufs=1) as p3, \
                     tc.tile_pool(name="ph3ps", bufs=1, space="PSUM") as psum3:
                    ps = psum3.tile([16, 128], dt.float32, tag="srcT")
                    nc.tensor.transpose(ps[:], src_col[:], ident[:])
                    srcT = p3.tile([16, 128], dt.float32, tag="srcTs")
                    nc.scalar.copy(srcT[:], ps[:])
                    srcTm = p3.tile([16, 128], dt.float32, tag="srcTm")
                    nc.scalar.activation(srcTm[:], ps[:], AF.Copy, bias=-L)
                    src_flat = p3.tile([1, N], dt.float32, tag="src_flat")
                    nc.sync.dma_start(
                        src_flat[:].rearrange("o (t p) -> o t p", t=NT), srcT[:])
                    srcm_flat = p3.tile([1, N], dt.float32, tag="srcm_flat")
                    nc.sync.dma_start(
                        srcm_flat[:].rearrange("o (t p) -> o t p", t=NT),
                        srcTm[:])
                    src_bc = pp.tile([128, N], dt.float32, tag="src_bc")
                    nc.gpsimd.partition_broadcast(src_bc[:], src_flat[:])
                    srcm_bc = pp.tile([128, N], dt.float32, tag="srcm_bc")
                    nc.gpsimd.partition_broadcast(srcm_bc[:], srcm_flat[:])

                if kphase == 3:
                    nc.sync.dma_start(d_out[0:128, 0:F_OUT], src_bc[:, 0:F_OUT])

                if kphase >= 4:
                    # ---- phase 4+5 per i-group: scores -> P -> PV ----
                    rz_col = pp.tile([128, NT], dt.float32, tag="rz_col")
                    with tc.tile_pool(name="pg", bufs=4) as sp, \
                         tc.tile_pool(name="pgP", bufs=1) as ppool, \
                         tc.tile_pool(name="pgo", bufs=4) as opool, \
                         tc.tile_pool(name="pgps", bufs=8,
                                      space="PSUM") as psum5:
                        for g in range(NG):
                            P = []
                            for tau in range(NT):
                                t, o = tau // 2, tau % 2
                                # u8 view of staging, m-ordered strided slice:
                                # byte = 512*B + 4*k + 2*hh + o ; B = 4g + bb
                                u8v = stag[t][:].bitcast(dt.uint8).rearrange(
                                    "p (b k hh o) -> p b hh k o",
                                    b=N // 256, k=128, hh=2, o=2)
                                s_pre = sp.tile([128, GW], dt.float32, tag="s_pre")
                                s_pre3 = s_pre[:].rearrange(
                                    "p (b hh k) -> p b hh k", b=BB, hh=2, k=128)
                                srcm3 = srcm_bc[:, GW * g:GW * (g + 1)].rearrange(
                                    "p (b hh k) -> p b hh k", b=BB, hh=2, k=128)
                                for hh in range(2):
                                    nc.vector.scalar_tensor_tensor(
                                        s_pre3[:, :, hh, :],
                                        u8v[:, BB * g:BB * (g + 1), hh, :, o], c200[:],
                                        srcm3[:, :, hh, :],
                                        op0=OP.mult, op1=OP.add)
                                if tau // CHG == g:
                                    lo = 128 * tau - GW * g
                                    nc.vector.copy_predicated(
                                        s_pre[:, lo:lo + 128], eye_u8[:],
                                        src_bc[:, 128 * tau:128 * tau + 128])
                                pt = ppool.tile([128, GW], dt.float32r, tag=f"P{tau}", name=f"P{tau}")
                                nc.scalar.activation(pt[:], s_pre[:], AF.Exp,
                                                     bias=dst_col[:, tau:tau + 1])
                                e2 = sp.tile([128, GW], dt.float32r, tag="e2")
                                nc.scalar.activation(e2[:], s_pre[:], AF.Exp,
                                                     bias=dst02_col[:, tau:tau + 1],
                                                     scale=0.2)
                                nc.vector.tensor_tensor(pt[:],
                                                        pt[:].bitcast(dt.float32),
                                                        e2[:].bitcast(dt.float32),
                                                        op=OP.max)
                                P.append(pt)

                            FH = ((512, F_OUT + 2), (0, 512))  # Z-half first
                            for fh, (f0, f1) in enumerate(FH):
                                nf = f1 - f0
                                for c in range(CHG):
                                    pso = psum5.tile([128, 512], dt.float32, tag="pso")
                                    for tau in range(NT):
                                        nc.tensor.matmul(
                                            pso[:, 0:nf],
                                            P[tau][:, 128 * c:128 * (c + 1)],
                                            hp[tau][:, f0:f1],
                                            start=(tau == 0), stop=(tau == NT - 1))
                                    ci = CHG * g + c
                                    if fh == 0:
                                        nc.vector.reciprocal(
                                            rz_col[:, ci:ci + 1], pso[:, 256:257])
                                    ob = opool.tile([128, 256 if fh == 0 else 512],
                                                    dt.float32, tag=f"ob{fh}")
                                    nc.scalar.activation(
                                        ob[:], pso[:, 0:ob.shape[1]], AF.Copy,
                                        scale=rz_col[:, ci:ci + 1])
                                    base = sig_off(ci)
                                    orows = d_out[base:base + 255:2,
                                                  f0:f0 + ob.shape[1]]
                                    nc.sync.dma_start(orows, ob[:])

    nc.compile()
    return nc


def _get_program(has_bias: bool):
    key = ("prog", has_bias)
    if key not in _CACHE:
        _CACHE[key] = _build(has_bias)
    return _CACHE[key]


def kernel(h, adj, w, a_src, a_dst, bias):
    from concourse.bass_utils import run_bass_kernel_spmd

    h = np.ascontiguousarray(np.asarray(h, dtype=np.float32))
    adj_u8 = np.ascontiguousarray(np.asarray(adj).astype(np.uint8))
    w = np.ascontiguousarray(np.asarray(w, dtype=np.float32))
    a_src = np.asarray(a_src, dtype=np.float32).reshape(H, F_OUT)
    a_dst = np.asarray(a_dst, dtype=np.float32).reshape(H, F_OUT)
    bias = np.asarray(bias, dtype=np.float32).reshape(F_OUT)
    has_bias = bool(np.any(bias))

    nc = _get_program(has_bias)

    in_maps = []
    for core in range(NCORES):
        b, hd = core // H, core % H
        m = {
            "h": h[b],
            "adj": adj_u8[b],
            "w": w[hd],
            "a_src": a_src[hd],
            "a_dst": a_dst[hd],
        }
        if has_bias:
            m["bias"] = bias
        in_maps.append(m)

    res = run_bass_kernel_spmd(nc, in_maps, list(range(NCORES)))
    out = np.empty((BS, H, N, F_OUT), dtype=np.float32)
    for core in range(NCORES):
        b, hd = core // H, core % H
        out[b, hd] = res.results[core]["out"]
    return out

